# revision 5
# baseline (speedup 1.0000x reference)
"""MoE-LoRA Linear kernel for 8x Trainium2 NeuronCores.

Math: out = x @ W^T + bias + sum_e gate[e] * (x @ A_e^T) @ B_e^T
  x [4,2048,4096], W [4096,4096], A [8,8,4096], B [8,4096,8].
  gate = softmax(router(expert_embed)) top-2 masked * scaling (per-task
  scalars: 8 numbers).

The gate is a per-task constant, so the whole LoRA term is a rank-64
update to W: the host folds W' = W + B @ diag(gate) @ A (one small
sgemm) and the device runs a pure GEMM: out = x @ W'^T + bias.

Device strategy (data-parallel over the 8192 tokens, 1024/core):
  - host pre-transposes and casts to bf16: xT [4096,1024] per core,
    W'T [4096,4096] replicated. bf16 halves HBM traffic and enables
    fast-weight-load on the PE; PSUM still accumulates fp32.
  - per core: x^T resident in SBUF (8.4 MB); W' streamed as [128,1024]
    o-pair tiles so each stationary x-tile load feeds TWO matmuls
    (halves LDWEIGHTS pressure on the PE). W is re-streamed for each
    token-half (2x W traffic; DMA has 40% headroom vs the PE).
  - fp32 bias rides on the DVE eviction add.
"""

import numpy as np

B_, S, D = 4, 2048, 4096
O = 4096
N_CORES = 8
TOKENS = B_ * S
T = TOKENS // N_CORES  # tokens per core
NUM_EXPERTS = 8
TOP_K = 2
SCALING = 16.0 / 64.0
R = 64  # total LoRA rank (8 experts x 8)

_BUILT = None


def _build():
    import concourse.bacc as bacc
    import concourse.mybir as mybir
    from concourse.bass import ts
    from concourse.tile import TileContext

    dt = mybir.dt
    f32 = dt.float32
    bf16 = dt.bfloat16
    P = 128
    DT = D // P          # 32 d-tiles
    OTILE = 512
    NOP = O // (2 * OTILE)   # 4 o-pairs
    TH = 2                   # token halves
    TQ = 4                   # token tiles per half

    nc = bacc.Bacc("TRN2", target_bir_lowering=False, debug=False)
    xT = nc.dram_tensor("xT", [D, T], bf16, kind="ExternalInput")
    wT = nc.dram_tensor("WT", [D, O], bf16, kind="ExternalInput")
    bias_d = nc.dram_tensor("BIAS", [1, O], f32, kind="ExternalInput")
    out = nc.dram_tensor("OUT", [T, O], f32, kind="ExternalOutput")

    with TileContext(nc) as tc:
        with (
            tc.tile_pool(name="resident", bufs=1) as res,
            tc.tile_pool(name="wpool", bufs=8) as wpool,
            tc.tile_pool(name="opool", bufs=10) as opool,
        ):
            x_sb = res.tile([P, DT, T], bf16, tag="x_sb")
            bias_sb = res.tile([P, O], f32, tag="bias_sb")

            # o-pair outer, token-half mid, d-tile inner. Each (d, tokhalf)
            # iteration: 4 stationary x-tiles, each feeding 2 matmuls (the
            # two o-tiles of the pair) -> 8 PSUM banks live.
            with tc.tile_pool(name="psum", bufs=8, space="PSUM") as pp:
                for opi in range(NOP):
                    first = opi == 0
                    last = opi == NOP - 1
                    for th in range(TH):
                        psums = [
                            [
                                pp.tile(
                                    [P, OTILE], f32, tag="pout",
                                    name=f"pout_{opi}_{th}_{t}_{j}",
                                )
                                for j in range(2)
                            ]
                            for t in range(TQ)
                        ]
                        for dti in range(DT):
                            if first and th == 0:
                                # split x per token-half: this half's d-tile
                                # on the fast path, the other half + bias on
                                # the scalar queue (needed ~55us later)
                                dsl = slice(dti * P, (dti + 1) * P)
                                nc.sync.dma_start(
                                    x_sb[:, dti, 0:T // 2], xT[dsl, 0:T // 2]
                                )
                                nc.scalar.dma_start(
                                    x_sb[:, dti, T // 2:T], xT[dsl, T // 2:T]
                                )
                                if dti == 8:
                                    nc.scalar.dma_start(
                                        bias_sb[:],
                                        bias_d[:].to_broadcast((P, O)),
                                    )
                            w_t = wpool.tile([P, 2 * OTILE], bf16, tag="w_t")
                            nc.sync.dma_start(
                                w_t[:],
                                wT[dti * P:(dti + 1) * P,
                                   opi * 2 * OTILE:(opi + 1) * 2 * OTILE],
                            )
                            for t in range(TQ):
                                tok = th * TQ + t
                                for j in range(2):
                                    nc.tensor.matmul(
                                        psums[t][j][:],
                                        lhsT=x_sb[:, dti, ts(tok, P)],
                                        rhs=w_t[:, ts(j, OTILE)],
                                        start=(dti == 0),
                                        stop=(dti == DT - 1),
                                    )
                        for t in range(TQ):
                            tok = th * TQ + t
                            for j in range(2):
                                osl = slice(
                                    (2 * opi + j) * OTILE,
                                    (2 * opi + j + 1) * OTILE,
                                )
                                o_t = opool.tile([P, OTILE], f32, tag="o_t")
                                nc.vector.tensor_add(
                                    out=o_t[:], in0=psums[t][j][:],
                                    in1=bias_sb[:, osl],
                                )
                                # final stores: spread over both HWDGE queues
                                eng = (
                                    nc.scalar
                                    if (last and th == 1 and (t + j) % 2 == 1)
                                    else nc.sync
                                )
                                eng.dma_start(out[ts(tok, P), osl], o_t[:])

    nc.compile()
    return nc


def _get_nc():
    global _BUILT
    if _BUILT is None:
        _BUILT = _build()
    return _BUILT


def _host_prep(x, W, bias, A, B, expert_embed, router_w):
    x = np.asarray(x, dtype=np.float32)
    W = np.asarray(W, dtype=np.float32)
    bias = np.asarray(bias, dtype=np.float32)
    A = np.asarray(A, dtype=np.float32)
    B = np.asarray(B, dtype=np.float32)
    expert_embed = np.asarray(expert_embed, dtype=np.float32)
    router_w = np.asarray(router_w, dtype=np.float32)

    # Router (per-task, 8 scalars)
    logits = (expert_embed[0] @ router_w.T).astype(np.float32)
    e = np.exp(logits - logits.max())
    probs = (e / e.sum()).astype(np.float32)
    sel = np.argsort(-probs, kind="stable")[:TOP_K]
    gate = np.zeros(NUM_EXPERTS, np.float32)
    gate[sel] = probs[sel] * np.float32(SCALING)

    import ml_dtypes

    # Fold the (per-task constant) gated LoRA into W:
    #   W' = W + sum_e gate_e * B_e @ A_e  -- a rank-64 update.
    Bcat = np.ascontiguousarray(B.transpose(1, 0, 2).reshape(O, R))
    Ascaled = (A * gate[:, None, None]).reshape(R, D)
    Wp = W + Bcat @ Ascaled

    WT = np.ascontiguousarray(Wp.T, dtype=ml_dtypes.bfloat16)
    BIAS = np.ascontiguousarray(bias.reshape(1, O), dtype=np.float32)

    xflat = x.reshape(TOKENS, D)
    in_maps = []
    for c in range(N_CORES):
        xt_shard = np.ascontiguousarray(
            xflat[c * T:(c + 1) * T, :].T, dtype=ml_dtypes.bfloat16
        )
        in_maps.append({"xT": xt_shard, "WT": WT, "BIAS": BIAS})
    return in_maps


def _execute(in_maps, trace=False, **kwargs):
    from concourse.bass_utils import run_bass_kernel_spmd

    nc = _get_nc()
    return run_bass_kernel_spmd(
        nc, in_maps, core_ids=list(range(N_CORES)), trace=trace, **kwargs
    )


def kernel(x, W, bias, A, B, expert_embed, router_w):
    in_maps = _host_prep(x, W, bias, A, B, expert_embed, router_w)
    res = _execute(in_maps, trace=False)
    out = np.concatenate([r["OUT"] for r in res.results], axis=0)
    return out.reshape(B_, S, O).astype(np.float32, copy=False)
